# revision 1
# baseline (speedup 1.0000x reference)
"""BitConv2d inference kernel for Trainium2 (8 NeuronCores, SPMD).

Problem: y = conv2d(x, w_q.float(), stride=1, pad=1) * s + bias
  x:    (32, 128, 56, 56) f32
  w_q:  (256, 128, 3, 3) ternary {-1,0,+1} (int8 or int32)
  s:    (256, 1, 1) f32
  bias: (256,) f32
  y:    (32, 256, 56, 56) f32

Strategy: data-parallel over batch (4 images per core). On each core the
conv is 9 shifted matmuls per output tile: x is laid out channel-major
[C_in=128 partitions, flat padded image] with row stride 57 (the single
zero column between consecutive rows serves as both right-pad of row r
and left-pad of row r+1), so the rhs for tap (kh, kw) is a contiguous
slice. Output tiles are [128 C_out-chunk, 8 rows * 57] PSUM
accumulations; scale+bias applied on ScalarE while dropping the pad
column per row; dense DMA out.

x is fed in fp16 (hi) with an optional exact residual pass: lo =
(x - fp16(x)) * 2^11 in fp16 against weights pre-scaled by 2^-11,
accumulated into the same PSUM group -> near-fp32 accuracy.

Each image is split into a top chunk (output blocks 0-3) and a bottom
chunk (blocks 4-6) so the first matmuls only wait for ~1MB of DMA, and
dummy warm-up matmuls keep the PE busy during that wait (HAM un-throttle
to 2.4 GHz costs ~3.4us of sustained activity).
"""

import os

import numpy as np

import concourse.bass as bass
import concourse.mybir as mybir
from concourse import bacc
from concourse.tile import TileContext

# Problem constants (hardcoded per contract)
N_IMG, C_IN, C_OUT, H, W = 32, 128, 256, 56, 56
N_CORES = 8
IMG_PER_CORE = N_IMG // N_CORES  # 4
S = W + 1  # 57: flat row stride; col 56 of row r == left pad of row r+1
ROWS_PER_BLK = 8
N_BLK = H // ROWS_PER_BLK  # 7
FREE = ROWS_PER_BLK * S  # 456  (<= 512 fp32 PSUM bank)
OUT_FREE = ROWS_PER_BLK * W  # 448
N_CHUNK = C_OUT // 128  # 2
LO_SCALE = 2048.0  # 2^11, exact in fp16

# padded flat image P[k], k = r*57 + c, r in 0..57 (58 rows: top/bottom pad)
# P[r*57+c] = x[r-1, c-1] for r,c in 1..56; P[r*57] = 0; + slack for the
# tail tap overshoot (block b, tap kh,kw reads [(b*8+kh)*57+kw : +456]).
P_ELEMS = 58 * S + 1  # 3307
TOP_BLKS = 4  # output blocks 0..3 read padded rows 0..33
TOP_ROWS = TOP_BLKS * ROWS_PER_BLK + 2  # 34
TOP_COLS = TOP_ROWS * S + 2  # 1940 (covers (26)*57+2+456)
BOT_ROW0 = TOP_BLKS * ROWS_PER_BLK  # padded row 32
BOT_COLS = P_ELEMS - BOT_ROW0 * S + 1  # 1484 (covers (18)*57+2+456)

# PASSES: 1 = fp16 hi only (~2e-4 rel err), 2 = hi + exact residual (~3e-7)
PASSES = int(os.environ.get("BITCONV_PASSES", "1"))
N_WARMUP = int(os.environ.get("BITCONV_WARMUP", "12"))
WARMUP_FREE = 256

f16 = mybir.dt.float16
f32 = mybir.dt.float32


def build_nc(passes: int) -> bacc.Bacc:
    nc = bacc.Bacc("TRN2", target_bir_lowering=False, debug=False)

    xtop = [
        nc.dram_tensor(f"xt{p}", [IMG_PER_CORE, C_IN, TOP_COLS], f16,
                       kind="ExternalInput").ap()
        for p in range(passes)
    ]
    xbot = [
        nc.dram_tensor(f"xb{p}", [IMG_PER_CORE, C_IN, BOT_COLS], f16,
                       kind="ExternalInput").ap()
        for p in range(passes)
    ]
    n_wblk = passes * N_CHUNK * 9
    wt = nc.dram_tensor("wt", [C_IN, n_wblk * 128], f16, kind="ExternalInput").ap()
    sv = nc.dram_tensor("sv", [128, N_CHUNK], f32, kind="ExternalInput").ap()
    bv = nc.dram_tensor("bv", [128, N_CHUNK], f32, kind="ExternalInput").ap()
    y = nc.dram_tensor("y", [IMG_PER_CORE, C_OUT, H, W], f32,
                       kind="ExternalOutput").ap()

    with TileContext(nc) as tc:
        with (
            tc.tile_pool(name="xpool", bufs=IMG_PER_CORE * passes) as xpool,
            tc.tile_pool(name="wpool", bufs=1) as wpool,
            tc.tile_pool(name="cpool", bufs=1) as cpool,
            tc.tile_pool(name="opool", bufs=6) as opool,
            tc.tile_pool(name="ppool", bufs=6, space="PSUM") as ppool,
            tc.tile_pool(name="wps", bufs=1, space="PSUM") as wps_pool,
        ):
            # scratch for PE warm-up (zeros; written before first DMA lands)
            wu = cpool.tile([128, WARMUP_FREE], f16, tag="wu")
            nc.vector.memset(wu[:, :], 0.0)
            wu_ps = wps_pool.tile([128, WARMUP_FREE], f32, tag="wups")

            # weights + first image chunks first: they gate the first matmul.
            # Weight blocks are laid out c-major (chunk 0 of every pass
            # first), and the DMA is split per chunk so only the chunk-0
            # slice gates the first PSUM tile.
            wt_t = wpool.tile([C_IN, n_wblk * 128], f16)
            half = passes * 9 * 128
            nc.sync.dma_start(out=wt_t[:, 0:half], in_=wt[:, 0:half])

            xt_t = [[None] * IMG_PER_CORE for _ in range(passes)]
            xb_t = [[None] * IMG_PER_CORE for _ in range(passes)]
            for i in range(IMG_PER_CORE):
                for p in range(passes):
                    t = xpool.tile([C_IN, TOP_COLS], f16, tag="xtop")
                    nc.sync.dma_start(out=t[:, :], in_=xtop[p][i])
                    xt_t[p][i] = t
                if i == 0:
                    nc.sync.dma_start(out=wt_t[:, half:], in_=wt[:, half:])
                for p in range(passes):
                    b = xpool.tile([C_IN, BOT_COLS], f16, tag="xbot")
                    nc.sync.dma_start(out=b[:, :], in_=xbot[p][i])
                    xb_t[p][i] = b
                if i == 0:
                    sv_t = cpool.tile([128, N_CHUNK], f32, tag="sv")
                    bv_t = cpool.tile([128, N_CHUNK], f32, tag="bv")
                    nc.sync.dma_start(out=sv_t[:, :], in_=sv[:, :])
                    nc.sync.dma_start(out=bv_t[:, :], in_=bv[:, :])

            # HAM warm-up: dummy matmuls on the zero scratch keep the PE
            # active while the first real DMAs are in flight.
            for _ in range(N_WARMUP):
                nc.tensor.matmul(wu_ps[:, :], wu[:, 0:128], wu[:, :],
                                 start=True, stop=True)

            nmm = passes * 9
            for i in range(IMG_PER_CORE):
                for c in range(N_CHUNK):
                    # one [128, H*W] staging tile per (image, chunk);
                    # flushed in two DMAs (blocks 0-3, blocks 4-6)
                    ot = opool.tile([128, N_BLK * OUT_FREE], f32, tag="ot")
                    for b in range(N_BLK):
                        top = b < TOP_BLKS
                        row0 = b * ROWS_PER_BLK - (0 if top else BOT_ROW0)
                        ps = ppool.tile([128, FREE], f32, tag="ps")
                        k = 0
                        for p in range(passes):
                            src = (xt_t if top else xb_t)[p][i]
                            for kh in range(3):
                                for kw in range(3):
                                    off = (row0 + kh) * S + kw
                                    blk = (c * passes + p) * 9 + kh * 3 + kw
                                    nc.tensor.matmul(
                                        ps[:, :],
                                        wt_t[:, blk * 128:(blk + 1) * 128],
                                        src[:, off:off + FREE],
                                        start=(k == 0),
                                        stop=(k == nmm - 1),
                                    )
                                    k += 1
                        ps3 = ps[:, :].rearrange(
                            "q (r c) -> q r c", r=ROWS_PER_BLK)[:, :, 0:W]
                        ot3 = ot[:, b * OUT_FREE:(b + 1) * OUT_FREE].rearrange(
                            "q (r c) -> q r c", r=ROWS_PER_BLK)
                        nc.scalar.activation(
                            ot3, ps3, mybir.ActivationFunctionType.Identity,
                            bias=bv_t[:, c:c + 1], scale=sv_t[:, c:c + 1])
                        if b == TOP_BLKS - 1:
                            nc.sync.dma_start(
                                out=y[i, c * 128:(c + 1) * 128,
                                      0:TOP_BLKS * ROWS_PER_BLK, :],
                                in_=ot[:, 0:TOP_BLKS * OUT_FREE])
                    nc.sync.dma_start(
                        out=y[i, c * 128:(c + 1) * 128,
                              TOP_BLKS * ROWS_PER_BLK:H, :],
                        in_=ot[:, TOP_BLKS * OUT_FREE:])

    nc.compile()
    return nc


def build_nc_raw(passes: int) -> bacc.Bacc:
    """Hand-synchronized variant: same dataflow as build_nc but with ~14
    explicit semaphores instead of Tile's per-tile sems, so the kernel
    head/tail overhead (preamble + end-of-kernel sem reset chain) shrinks."""
    from contextlib import ExitStack

    nc = bacc.Bacc("TRN2", target_bir_lowering=False, debug=False)

    xtop = [
        nc.dram_tensor(f"xt{p}", [IMG_PER_CORE, C_IN, TOP_COLS], f16,
                       kind="ExternalInput").ap()
        for p in range(passes)
    ]
    xbot = [
        nc.dram_tensor(f"xb{p}", [IMG_PER_CORE, C_IN, BOT_COLS], f16,
                       kind="ExternalInput").ap()
        for p in range(passes)
    ]
    n_wblk = passes * N_CHUNK * 9
    wt = nc.dram_tensor("wt", [C_IN, n_wblk * 128], f16, kind="ExternalInput").ap()
    sv = nc.dram_tensor("sv", [128, N_CHUNK], f32, kind="ExternalInput").ap()
    bv = nc.dram_tensor("bv", [128, N_CHUNK], f32, kind="ExternalInput").ap()
    y = nc.dram_tensor("y", [IMG_PER_CORE, C_OUT, H, W], f32,
                       kind="ExternalOutput").ap()

    # static SBUF
    xt_t = [[nc.alloc_sbuf_tensor(f"sxt{p}_{i}", [C_IN, TOP_COLS], f16).ap()
             for i in range(IMG_PER_CORE)] for p in range(passes)]
    xb_t = [[nc.alloc_sbuf_tensor(f"sxb{p}_{i}", [C_IN, BOT_COLS], f16).ap()
             for i in range(IMG_PER_CORE)] for p in range(passes)]
    wt_t = nc.alloc_sbuf_tensor("swt", [C_IN, n_wblk * 128], f16).ap()
    sv_t = nc.alloc_sbuf_tensor("ssv", [128, N_CHUNK], f32).ap()
    bv_t = nc.alloc_sbuf_tensor("sbv", [128, N_CHUNK], f32).ap()
    wu = nc.alloc_sbuf_tensor("swu", [128, WARMUP_FREE], f16).ap()
    N_OT = 2
    ot_t = [nc.alloc_sbuf_tensor(f"sot{j}", [128, N_BLK * OUT_FREE], f32).ap()
            for j in range(N_OT)]
    # PSUM: 6 compute slots + 1 warm-up bank, each exactly one 2KB bank
    N_PS = 6
    ps_t = [nc.alloc_psum_tensor(f"ps{j}", [128, 512], f32).ap()
            for j in range(N_PS)]
    wu_ps = nc.alloc_psum_tensor("wups", [128, 512], f32).ap()

    half = passes * 9 * 128
    n_groups = IMG_PER_CORE * N_CHUNK  # (image, chunk) groups of N_BLK tiles
    XIN = 16 * passes

    with ExitStack() as ctx:
        s_wt = ctx.enter_context(nc.semaphore("s_wt"))
        s_wtb = ctx.enter_context(nc.semaphore("s_wtb"))
        s_wt2 = ctx.enter_context(nc.semaphore("s_wt2"))
        s_x0ab = ctx.enter_context(nc.semaphore("s_x0ab"))
        s_cst = ctx.enter_context(nc.semaphore("s_cst"))
        s_wu = ctx.enter_context(nc.semaphore("s_wu"))
        s_mm = ctx.enter_context(nc.semaphore("s_mm"))
        s_act = ctx.enter_context(nc.semaphore("s_act"))
        # one per ot slot so every wait is a full total (sound under
        # interleaved per-queue increments)
        s_out = [ctx.enter_context(nc.semaphore(f"s_out{j}"))
                 for j in range(N_OT)]
        s_xt = [ctx.enter_context(nc.semaphore(f"s_xt{i}"))
                for i in range(IMG_PER_CORE)]
        s_xb = [ctx.enter_context(nc.semaphore(f"s_xb{i}"))
                for i in range(IMG_PER_CORE)]
        s_x0b = ctx.enter_context(nc.semaphore("s_x0b"))
        block = ctx.enter_context(nc.Block())

        @block.vector
        def _(eng):
            eng.memset(wu[:, :], 0.0).then_inc(s_wu, 1)

        # image-0 top is split so the first matmuls start after ~0.25MB
        xt0_s1 = 2 * S + 2 + FREE  # 572: block 0
        xt0_s2 = (ROWS_PER_BLK + 2) * S + 2 + FREE  # 1028: block 1
        wt_s1 = 3 * 128  # first 3 weight blocks gate matmuls 0-2

        @block.scalar
        def _(eng):
            # input DMAs on the ACT HWDGE ring, in parallel with the SP
            # ring that carries the output flushes. Critical prefix first:
            # first 3 chunk-0 weight blocks + image-0 rows 0-9.
            eng.dma_start(out=wt_t[:, 0:wt_s1], in_=wt[:, 0:wt_s1]).then_inc(s_wt, 16)
            for p in range(passes):
                eng.dma_start(out=xt_t[p][0][:, 0:xt0_s1],
                              in_=xtop[p][0][:, 0:xt0_s1]).then_inc(s_xt[0], 16)
            eng.dma_start(out=wt_t[:, wt_s1:half], in_=wt[:, wt_s1:half]).then_inc(s_wtb, 16)
            for p in range(passes):
                eng.dma_start(out=xt_t[p][0][:, xt0_s1:xt0_s2],
                              in_=xtop[p][0][:, xt0_s1:xt0_s2]).then_inc(s_x0ab, 16)
            # throttle: only the block-0/1 critical set shares SDMA
            # round-robin bandwidth until it has landed
            eng.wait_ge(s_x0ab, XIN)
            for p in range(passes):
                eng.dma_start(out=xt_t[p][0][:, xt0_s2:],
                              in_=xtop[p][0][:, xt0_s2:]).then_inc(s_x0b, 16)
            eng.wait_ge(s_x0b, XIN)
            for p in range(passes):
                eng.dma_start(out=xb_t[p][0][:, :], in_=xbot[p][0]).then_inc(s_xb[0], 16)
            eng.dma_start(out=wt_t[:, half:], in_=wt[:, half:]).then_inc(s_wt2, 16)
            eng.dma_start(out=sv_t[:, :], in_=sv[:, :]).then_inc(s_cst, 16)
            eng.dma_start(out=bv_t[:, :], in_=bv[:, :]).then_inc(s_cst, 16)
            for i in range(1, IMG_PER_CORE):
                for p in range(passes):
                    eng.dma_start(out=xt_t[p][i][:, :], in_=xtop[p][i]).then_inc(s_xt[i], 16)
                for p in range(passes):
                    eng.dma_start(out=xb_t[p][i][:, :], in_=xbot[p][i]).then_inc(s_xb[i], 16)

        @block.sync
        def _(eng):
            # output flushes: 3 per (image, chunk) group (after ACTs 4, 6, 7)
            # so the final flush after the last ACT is a single small block
            flush_blks = [(0, TOP_BLKS), (TOP_BLKS, N_BLK - 1), (N_BLK - 1, N_BLK)]
            for g in range(n_groups):
                i, c = divmod(g, N_CHUNK)
                ot = ot_t[g % N_OT]
                for b0, b1 in flush_blks:
                    eng.wait_ge(s_act, g * N_BLK + b1)
                    eng.dma_start(
                        out=y[i, c * 128:(c + 1) * 128,
                              b0 * ROWS_PER_BLK:b1 * ROWS_PER_BLK, :],
                        in_=ot[:, b0 * OUT_FREE:b1 * OUT_FREE]
                    ).then_inc(s_out[g % N_OT], 16)

        @block.gpsimd
        def _(eng):
            # finalizer: wait for all output flushes before the exit barrier
            for j in range(N_OT):
                nj = (n_groups - j + N_OT - 1) // N_OT  # groups on slot j
                eng.wait_ge(s_out[j], nj * 48)

        @block.tensor
        def _(eng):
            eng.wait_ge(s_wu, 1)
            for _ in range(N_WARMUP):
                nc.tensor.matmul(wu_ps[:, 0:WARMUP_FREE], wu[:, 0:128], wu[:, :],
                                 start=True, stop=True)
            eng.wait_ge(s_wt, 16)
            tile_idx = 0
            for g in range(n_groups):
                i, c = divmod(g, N_CHUNK)
                if c == 0:
                    eng.wait_ge(s_xt[i], XIN)
                if g == 1:
                    eng.wait_ge(s_wt2, 16)
                for b in range(N_BLK):
                    if g == 0 and b == 1:
                        eng.wait_ge(s_x0ab, XIN)
                    if g == 0 and b == 2:
                        eng.wait_ge(s_x0b, XIN)
                    if c == 0 and b == TOP_BLKS:
                        eng.wait_ge(s_xb[i], XIN)
                    if tile_idx >= N_PS:
                        # tile reuses the slot last used by tile_idx - N_PS
                        eng.wait_ge(s_act, tile_idx - N_PS + 1)
                    top = b < TOP_BLKS
                    row0 = b * ROWS_PER_BLK - (0 if top else BOT_ROW0)
                    ps = ps_t[tile_idx % N_PS]
                    k = 0
                    for p in range(passes):
                        src = (xt_t if top else xb_t)[p][i]
                        for kh in range(3):
                            for kw in range(3):
                                if g == 0 and b == 0 and k == 3:
                                    eng.wait_ge(s_wtb, 16)
                                off = (row0 + kh) * S + kw
                                blk = (c * passes + p) * 9 + kh * 3 + kw
                                mmi = nc.tensor.matmul(
                                    ps[:, 0:FREE],
                                    wt_t[:, blk * 128:(blk + 1) * 128],
                                    src[:, off:off + FREE],
                                    start=(k == 0),
                                    stop=(k == passes * 9 - 1),
                                )
                                k += 1
                    mmi.then_inc(s_mm, 1)
                    tile_idx += 1

        @block.scalar
        def _(eng):
            eng.wait_ge(s_cst, 32)
            tile_idx = 0
            for g in range(n_groups):
                ot = ot_t[g % N_OT]
                if g >= N_OT:
                    # ot slot reusable once the previous user's flushes done
                    eng.wait_ge(s_out[g % N_OT], ((g - N_OT) // N_OT + 1) * 48)
                for b in range(N_BLK):
                    ps = ps_t[tile_idx % N_PS]
                    eng.wait_ge(s_mm, tile_idx + 1)
                    ps3 = ps[:, 0:FREE].rearrange(
                        "q (r c) -> q r c", r=ROWS_PER_BLK)[:, :, 0:W]
                    ot3 = ot[:, b * OUT_FREE:(b + 1) * OUT_FREE].rearrange(
                        "q (r c) -> q r c", r=ROWS_PER_BLK)
                    eng.activation(
                        ot3, ps3, mybir.ActivationFunctionType.Identity,
                        bias=bv_t[:, g % N_CHUNK:g % N_CHUNK + 1],
                        scale=sv_t[:, g % N_CHUNK:g % N_CHUNK + 1],
                    ).then_inc(s_act, 1)
                    tile_idx += 1

        # exit: one all-engine barrier, then reset DMA/sem state so the
        # NEFF can be re-executed
        nc.all_engine_barrier()
        nc.gpsimd.dma_reset()
        nc.gpsimd.sem_clear(nc._kernel_sem_range)

    nc.compile()
    return nc


def prep_inputs(x, w_q, s, bias, passes: int):
    """Full inputs -> list of 8 per-core in_maps (numpy)."""
    x = np.asarray(x, dtype=np.float32)
    wq = np.asarray(w_q).astype(np.float32)
    s = np.asarray(s, dtype=np.float32).reshape(C_OUT)
    bias = np.asarray(bias, dtype=np.float32).reshape(C_OUT)

    # x -> fp16 hi (+ scaled fp16 residual)
    x_hi = x.astype(np.float16)
    parts = [x_hi]
    if passes == 2:
        x_lo = ((x - x_hi.astype(np.float32)) * LO_SCALE).astype(np.float16)
        parts.append(x_lo)

    in_maps = [dict() for _ in range(N_CORES)]
    for p, xp in enumerate(parts):
        # padded flat layout P: 58 rows of stride 57 (+ tail slack)
        buf = np.zeros((N_CORES, IMG_PER_CORE, C_IN, P_ELEMS + 3), np.float16)
        v = np.lib.stride_tricks.as_strided(
            buf[:, :, :, S + 1:],  # row r=1, col c=1
            shape=(N_CORES, IMG_PER_CORE, C_IN, H, W),
            strides=buf.strides[:3] + (buf.strides[3] * S, buf.strides[3]),
        )
        v[:] = xp.reshape(N_CORES, IMG_PER_CORE, C_IN, H, W)
        for core in range(N_CORES):
            in_maps[core][f"xt{p}"] = np.ascontiguousarray(
                buf[core, :, :, :TOP_COLS])
            in_maps[core][f"xb{p}"] = np.ascontiguousarray(
                buf[core, :, :, BOT_ROW0 * S:BOT_ROW0 * S + BOT_COLS])

    # weights: block index (c*passes + p)*9 + kh*3 + kw, each [C_IN, 128]
    # with wt[p_cin, blk, m] = w_q[c*128+m, p_cin, kh, kw] (* lo scale)
    w5 = np.transpose(wq.reshape(N_CHUNK, 128, C_IN, 3, 3), (2, 0, 3, 4, 1))
    w5 = np.ascontiguousarray(w5).reshape(C_IN, N_CHUNK, 9 * 128)
    scales = [1.0] if passes == 1 else [1.0, 1.0 / LO_SCALE]
    wt = np.concatenate(
        [w5[:, c] * sc for c in range(N_CHUNK) for sc in scales],
        axis=1).astype(np.float16)

    sv = np.ascontiguousarray(s.reshape(N_CHUNK, 128).T)
    bv = np.ascontiguousarray(bias.reshape(N_CHUNK, 128).T)
    for core in range(N_CORES):
        in_maps[core]["wt"] = wt
        in_maps[core]["sv"] = sv
        in_maps[core]["bv"] = bv
    return in_maps


_NC_CACHE: dict[tuple[int, bool], bacc.Bacc] = {}

RAW = bool(int(os.environ.get("BITCONV_RAW", "1")))


def get_nc(passes: int, raw: bool | None = None) -> bacc.Bacc:
    raw = RAW if raw is None else raw
    key = (passes, raw)
    if key not in _NC_CACHE:
        _NC_CACHE[key] = (build_nc_raw if raw else build_nc)(passes)
    return _NC_CACHE[key]


def run(inputs, trace: bool = False, passes: int = PASSES, **run_kwargs):
    """Returns (full_output, BassKernelResults)."""
    from concourse.bass_utils import run_bass_kernel_spmd

    nc = get_nc(passes)
    in_maps = prep_inputs(**inputs, passes=passes)
    res = run_bass_kernel_spmd(nc, in_maps, list(range(N_CORES)),
                               trace=trace, **run_kwargs)
    out = np.concatenate([np.asarray(res.results[i]["y"])
                          for i in range(N_CORES)], axis=0)
    return out, res


def kernel(**inputs) -> np.ndarray:
    out, _ = run(inputs)
    return out



# revision 29
# speedup vs baseline: 1.4064x; 1.4064x over previous
"""BitConv2d inference kernel for Trainium2 (8 NeuronCores, SPMD).

Problem: y = conv2d(x, w_q.float(), stride=1, pad=1) * s + bias
  x:    (32, 128, 56, 56) f32
  w_q:  (256, 128, 3, 3) ternary {-1,0,+1} (int8 or int32)
  s:    (256, 1, 1) f32
  bias: (256,) f32
  y:    (32, 256, 56, 56) f32

Strategy: data-parallel over batch (4 images per core) + Winograd F(2,3)
along H. The 3 kh-taps collapse into 4 host-precomputed fp16 "streams"
per image (S1..S4 = +/- combinations of adjacent padded rows), so each
pair of output rows needs only 4 matmul products instead of 6:

  bank1 = M1 = sum_kw V1[kw]^T S1(shift kw)      (3 matmuls)
  bank2 = M2                                      (3)
  bank3 = -M3  (V3 pre-negated on host)           (3)
  bank4 = M4                                      (3)
  even rows E = M1 + M2 + M3 + b,  odd rows O = M2 - M3 - M4 + b

12 matmuls per 16 output rows vs 18 direct = 1.5x fewer PE cycles.
The kw taps reuse the baseline's stride-57 shifted-slice trick (stream
rows are 57 wide; the zero column between rows is the shared pad).

Combine per group of 7 row-pairs (free dim 399 = 7x57, one PSUM bank):
  ACT: C2 = Id(bank2 + bias) -> fp16   (bias rides into both parities)
       B  = Id([bank3; bank4])  -> fp16, rows interleaved
  DVE: T(even) = bank1 + C2, T(odd) = bank3 + C2   (rows interleaved)
       stage   = T - B        (dense fp16 2x op; E = T-(-M3), O = T-M4)
All junk pad columns are dropped at the PSUM->SBUF boundary, so the
staging tile and the output DMA are fully contiguous (14 rows x 224B
per partition per group), line-rate both sides.

Scales s are folded into the transformed weights on the host.
Per-core per-group engine budget @2.4GHz: PE 1995ns, DVE ~1700ns,
ACT ~1570ns -> PE-bound, ~32 groups ~= 67-70us (vs 114.5us baseline).
"""

import os
from contextlib import ExitStack

import numpy as np

import concourse.bass as bass
import concourse.mybir as mybir
from concourse import bacc

# Problem constants (hardcoded per contract)
N_IMG, C_IN, C_OUT, H, W = 32, 128, 256, 56, 56
N_CORES = 8
IMG = N_IMG // N_CORES  # 4 images per core
IMG_PER_CORE = IMG  # test.py compat
PASSES = 1  # test.py compat (no residual pass needed: rel err ~1e-3)
N_CH = C_OUT // 128  # 2 cout chunks

SROW = 57  # stream row stride (56 cols + shared pad col)
NPAIR = 28  # output row pairs per image
GQ = 7  # row pairs per group
NGRP = NPAIR // GQ  # 4 groups per (img, chunk)
SFREE = GQ * SROW  # 399, matmul free dim (<= 512 fp32 PSUM bank)
SCOLS = NPAIR * SROW + 8  # 1604: stream cols + tap-overshoot slack
STG = 2 * GQ * W  # 784: staging elems per group (14 rows x 56)
IMG_RUN = int(os.environ.get("WINO_IMGS", "4"))  # debug: images to process
N_GRP = int(os.environ.get("WINO_GRPS", str(IMG_RUN * N_CH * NGRP)))
N_WU = int(os.environ.get("WINO_WARMUP", "12"))
KO = set(os.environ.get("WINO_KO", "").split(","))  # debug knockouts
N_WARMUP = 12
WU_FREE = 256

f16 = mybir.dt.float16
f32 = mybir.dt.float32


def build_nc() -> bacc.Bacc:
    nc = bacc.Bacc("TRN2", target_bir_lowering=False, debug=False)

    xs = nc.dram_tensor("xs", [IMG, 4, C_IN, SCOLS], f16,
                        kind="ExternalInput").ap()
    wt = nc.dram_tensor("wt", [C_IN, 24 * 128], f16, kind="ExternalInput").ap()
    bv = nc.dram_tensor("bv", [128, N_CH], f32, kind="ExternalInput").ap()
    y = nc.dram_tensor("y", [IMG, C_OUT, H, W], f32, kind="ExternalOutput").ap()

    # static SBUF
    xs_t = [nc.alloc_sbuf_tensor(f"sxs{i}", [C_IN, 4 * SCOLS], f16).ap()
            for i in range(IMG)]
    wt_t = nc.alloc_sbuf_tensor("swt", [C_IN, 24 * 128], f16).ap()
    bv_t = nc.alloc_sbuf_tensor("sbv", [128, N_CH], f32).ap()
    wu = nc.alloc_sbuf_tensor("swu", [128, WU_FREE], f16).ap()
    scr = nc.alloc_sbuf_tensor("sscr", [128, 16], f16).ap()
    scr34 = nc.alloc_sbuf_tensor("sscr34", [128, STG], f32).ap()
    c2_t = [nc.alloc_sbuf_tensor(f"sc2{h}", [128, GQ * W], f16).ap()
            for h in range(2)]
    t_t = [nc.alloc_sbuf_tensor(f"st{h}", [128, STG], f16).ap()
           for h in range(2)]
    b_t = [nc.alloc_sbuf_tensor(f"sb{h}", [128, STG], f16).ap()
           for h in range(2)]
    st_t = [nc.alloc_sbuf_tensor(f"sst{j}", [128, NGRP * STG], f32).ap()
            for j in range(2)]
    ps = nc.alloc_psum_tensor("ps", [128, 4096], f32).ap()

    def bank_mm(h, m):  # matmul target: one PSUM bank, dense
        base = (4 * h + m) * 512
        return ps[:, base:base + SFREE]

    def bank_rd(h, m):  # junk-column-free read view [p, 7, 56]
        base = (4 * h + m) * 512
        return ps[:, base:base + SFREE].rearrange(
            "p (r c) -> p r c", c=SROW)[:, :, 0:W]

    def b34_in(h):  # banks 3,4 as [p, 2, 7, 56]
        base = (4 * h + 2) * 512
        return ps[:, base:base + 1024].rearrange(
            "p (b x) -> p b x", b=2)[:, :, 0:SFREE].rearrange(
            "p b (r c) -> p b r c", c=SROW)[:, :, :, 0:W]

    def il_view(tile):  # [128, 784] -> row-interleaved [p, parity, 7, 56]
        return tile.rearrange("p (r t c) -> p t r c", t=2, c=W)

    def c2_view(h):
        return c2_t[h].rearrange("p (r c) -> p r c", c=W)

    wu_ps = ps[:, 7 * 512:7 * 512 + WU_FREE]

    Id = mybir.ActivationFunctionType.Identity

    def gidx(g):
        return g // 8, (g // 4) % 2, g % 4, g % 2  # img, chunk, grp, half

    with ExitStack() as ctx:
        s_wu = ctx.enter_context(nc.semaphore("s_wu"))
        s_wt = [ctx.enter_context(nc.semaphore(f"s_wt{k}")) for k in range(3)]
        s_x0p = [ctx.enter_context(nc.semaphore(f"s_x0p{m}")) for m in range(4)]
        s_x0b = ctx.enter_context(nc.semaphore("s_x0b"))
        s_x = [None] + [ctx.enter_context(nc.semaphore(f"s_x{i}"))
                        for i in range(1, IMG)]
        s_cst = ctx.enter_context(nc.semaphore("s_cst"))
        s_mm = ctx.enter_context(nc.semaphore("s_mm"))
        s_c2 = ctx.enter_context(nc.semaphore("s_c2"))
        s_b = ctx.enter_context(nc.semaphore("s_b"))
        s_d2 = ctx.enter_context(nc.semaphore("s_d2"))
        s_d34 = ctx.enter_context(nc.semaphore("s_d34"))
        s_out = [ctx.enter_context(nc.semaphore(f"s_out{j}"))
                 for j in range(2)]
        block = ctx.enter_context(nc.Block())

        @block.scalar
        def _(eng):
            # input DMAs on the ACT HWDGE ring; critical prefix first.
            # weight blocks ordered (chunk, m, kw); group 0 consumes
            # m-banks in order, so stage the DMAs to match.
            eng.dma_start(out=wt_t[:, 0:384], in_=wt[:, 0:384]).then_inc(s_wt[0], 16)
            for m in range(4):
                eng.dma_start(out=xs_t[0][:, m * SCOLS:m * SCOLS + 456],
                              in_=xs[0, m][:, 0:456]).then_inc(s_x0p[m], 16)
            eng.dma_start(out=wt_t[:, 384:1536], in_=wt[:, 384:1536]).then_inc(s_wt[1], 16)
            eng.dma_start(out=bv_t[:, :], in_=bv[:, :]).then_inc(s_cst, 16)
            for m in range(4):
                eng.dma_start(out=xs_t[0][:, m * SCOLS + 456:(m + 1) * SCOLS],
                              in_=xs[0, m][:, 456:SCOLS]).then_inc(s_x0b, 16)
            eng.dma_start(out=wt_t[:, 1536:3072], in_=wt[:, 1536:3072]).then_inc(s_wt[2], 16)
            for i in range(1, IMG):
                for m in range(4):
                    eng.dma_start(out=xs_t[i][:, m * SCOLS:(m + 1) * SCOLS],
                                  in_=xs[i, m]).then_inc(s_x[i], 16)

            # dummy activation: forces the Identity table load (~2.7us)
            # to happen during the initial DMA window
            if N_WU:
                eng.wait_ge(s_wu, 1)
                eng.activation(scr[:, :], wu[:, 0:16], Id)
            eng.wait_ge(s_cst, 16)

            for g in range(N_GRP):
                i, c, q, h = gidx(g)
                eng.wait_ge(s_mm, 4 * g + 2)
                if g >= 2:
                    eng.wait_ge(s_d2, 2 * g - 2)  # c2 tile reuse
                bias = 0.0 if "c2f" in KO else bv_t[:, c:c + 1]
                eng.activation(c2_view(h), bank_rd(h, 1), Id,
                               bias=bias).then_inc(s_c2, 1)
                eng.wait_ge(s_mm, 4 * g + 4)
                if g >= 2:
                    eng.wait_ge(s_d34, g - 1)  # b tile reuse
                if "b34" in KO:
                    eng.activation(il_view(b_t[h])[:, 0], bank_rd(h, 2),
                                   Id)
                    eng.activation(il_view(b_t[h])[:, 1], bank_rd(h, 3),
                                   Id).then_inc(s_b, 1)
                else:
                    eng.activation(il_view(b_t[h]), b34_in(h), Id).then_inc(s_b, 1)

        @block.vector
        def _(eng):
            eng.memset(wu[:, :], 0.0).then_inc(s_wu, 1)
            for g in range(N_GRP):
                i, c, q, h = gidx(g)
                # C2(g) done implies banks 1,2 of g are done
                eng.wait_ge(s_c2, g + 1)
                if g >= 2:
                    eng.wait_ge(s_d34, g - 1)  # t tile reuse (WAR)
                if "op12" in KO:
                    t0 = t_t[h][:, 0:GQ * W].rearrange("p (r c) -> p r c", c=W)
                    t1 = t_t[h][:, GQ * W:STG].rearrange("p (r c) -> p r c", c=W)
                else:
                    t0 = il_view(t_t[h])[:, 0]
                    t1 = il_view(t_t[h])[:, 1]
                eng.tensor_tensor(t0, bank_rd(h, 0), c2_view(h),
                                  op=mybir.AluOpType.add).then_inc(s_d2, 1)
                # odd-row partial from B3's fp16 copy of bank3 (-M3), not
                # the PSUM bank itself: each PSUM bank must have exactly
                # one reader engine (ACT+DVE same-bank access collides)
                eng.wait_ge(s_b, g + 1)
                eng.tensor_tensor(t1, il_view(b_t[h])[:, 0], c2_view(h),
                                  op=mybir.AluOpType.add).then_inc(s_d2, 1)
                eng.wait_ge(s_d2, 2 * g + 2)  # own op1/op2 writeback drained
                if g >= 8:
                    eng.wait_ge(s_out[c], 64 * (g // 8))  # staging reuse
                o34 = (scr34[:, :] if "op34" in KO
                       else st_t[c][:, q * STG:(q + 1) * STG])
                eng.tensor_tensor(o34, t_t[h][:, :], b_t[h][:, :],
                                  op=mybir.AluOpType.subtract).then_inc(s_d34, 1)

        @block.sync
        def _(eng):
            for g in range(N_GRP):
                i, c, q, h = gidx(g)
                eng.wait_ge(s_d34, g + 1)
                if "flush" in KO:
                    eng.nop().then_inc(s_out[c], 16)
                else:
                    eng.dma_start(
                        out=y[i, c * 128:(c + 1) * 128, q * 14:(q + 1) * 14, :],
                        in_=st_t[c][:, q * STG:(q + 1) * STG],
                    ).then_inc(s_out[c], 16)

        @block.gpsimd
        def _(eng):
            nf = [sum(1 for g in range(N_GRP) if (g // 4) % 2 == c)
                  for c in range(2)]
            for c in range(2):
                if nf[c]:
                    eng.wait_ge(s_out[c], 16 * nf[c])

        @block.tensor
        def _(eng):
            eng.wait_ge(s_wu, 1)
            for _ in range(N_WU):
                nc.tensor.matmul(wu_ps, wu[:, 0:128], wu[:, :],
                                 start=True, stop=True)
            for g in range(N_GRP):
                i, c, q, h = gidx(g)
                if g == 1:
                    eng.wait_ge(s_x0b, 64)
                if g == 4:
                    eng.wait_ge(s_wt[2], 16)
                if g >= 8 and g % 8 == 0:
                    eng.wait_ge(s_x[i], 64)
                if g >= 2:
                    eng.wait_ge(s_d2, 2 * g - 2)
                    eng.wait_ge(s_b, g - 1)
                for m in range(4):
                    if g == 0:
                        if m == 0:
                            eng.wait_ge(s_wt[0], 16)
                        elif m == 1:
                            eng.wait_ge(s_wt[1], 16)
                        eng.wait_ge(s_x0p[m], 16)
                    blk = (c * 4 + m) * 3
                    off = m * SCOLS + q * SFREE
                    mi = None
                    for kw in range(3):
                        mi = nc.tensor.matmul(
                            bank_mm(h, m),
                            wt_t[:, (blk + kw) * 128:(blk + kw + 1) * 128],
                            xs_t[i][:, off + kw:off + kw + SFREE],
                            start=(kw == 0),
                            stop=(kw == 2),
                        )
                    mi.then_inc(s_mm, 1)

        nc.all_engine_barrier()
        nc.gpsimd.dma_reset()
        nc.gpsimd.sem_clear(nc._kernel_sem_range)

    nc.compile()
    return nc


def prep_inputs(x, w_q, s, bias, passes=None):
    """Full inputs -> list of 8 per-core in_maps (numpy)."""
    x = np.asarray(x, dtype=np.float32)
    wq = np.asarray(w_q).astype(np.float32)
    s = np.asarray(s, dtype=np.float32).reshape(C_OUT)
    bias = np.asarray(bias, dtype=np.float32).reshape(C_OUT)

    # Winograd F(2,3) row streams from the zero-padded image P[58, 57]
    # (P[p, q] = x[p-1, q-1]; row 0 / col 0 are the top/left pad, the
    # bottom pad is row 57, the right pad is col 0 of the next row).
    P = np.zeros((N_IMG, C_IN, H + 2, SROW), np.float32)
    P[:, :, 1:H + 1, 1:W + 1] = x.reshape(N_IMG, C_IN, H, W)
    S = np.stack([
        P[:, :, 0:56:2] - P[:, :, 2:58:2],  # S1
        P[:, :, 1:57:2] + P[:, :, 2:58:2],  # S2
        P[:, :, 2:58:2] - P[:, :, 1:57:2],  # S3
        P[:, :, 1:57:2] - P[:, :, 3:58:2],  # S4 (row 57 is the bottom pad)
    ], axis=1)  # [N, 4, C_IN, 28, 57]
    xs = np.zeros((N_IMG, 4, C_IN, SCOLS), np.float16)
    xs[:, :, :, :NPAIR * SROW] = S.reshape(N_IMG, 4, C_IN, NPAIR * SROW)
    xs = xs.reshape(N_CORES, IMG, 4, C_IN, SCOLS)

    # transformed weights, scale folded, V3 negated (bank3 = -M3),
    # blocks ordered (chunk, m, kw), each [C_IN, 128] pre-transposed
    w = wq * s[:, None, None, None]  # [256, 128, 3, 3]
    g0, g1, g2 = w[:, :, 0, :], w[:, :, 1, :], w[:, :, 2, :]
    V = np.stack([
        g0,
        (g0 + g1 + g2) * 0.5,
        -(g0 - g1 + g2) * 0.5,
        g2,
    ], axis=0)  # [4, 256, C_IN, 3]
    V = V.reshape(4, N_CH, 128, C_IN, 3)
    wt = np.ascontiguousarray(np.transpose(V, (3, 1, 0, 4, 2))).reshape(
        C_IN, 24 * 128).astype(np.float16)

    bv = np.ascontiguousarray(bias.reshape(N_CH, 128).T)

    in_maps = []
    for core in range(N_CORES):
        in_maps.append({"xs": np.ascontiguousarray(xs[core]),
                        "wt": wt, "bv": bv})
    return in_maps


_NC_CACHE: dict[str, bacc.Bacc] = {}


def get_nc(*_args, **_kwargs) -> bacc.Bacc:
    if "nc" not in _NC_CACHE:
        _NC_CACHE["nc"] = build_nc()
    return _NC_CACHE["nc"]


def run(inputs, trace: bool = False, **run_kwargs):
    """Returns (full_output, BassKernelResults)."""
    from concourse.bass_utils import run_bass_kernel_spmd

    run_kwargs.pop("passes", None)
    nc = get_nc()
    in_maps = prep_inputs(**inputs)
    res = run_bass_kernel_spmd(nc, in_maps, list(range(N_CORES)),
                               trace=trace, **run_kwargs)
    out = np.concatenate([np.asarray(res.results[i]["y"])
                          for i in range(N_CORES)], axis=0)
    return out, res


def kernel(**inputs) -> np.ndarray:
    out, _ = run(inputs)
    return out
